# revision 21
# baseline (speedup 1.0000x reference)
"""Multi-class 3D DICE loss on 8 Trainium2 NeuronCores.

Data-parallel over the subject (batch) axis: core b reduces subject b's
[C=4, 64, 128, 128] volumes to a single per-subject loss scalar; the host
averages the 8 scalars.

Per-core layout: each input tensor is viewed as [128, 32768] where
partition q = c*32 + p (class c in partition block [32c, 32c+32)).
Per chunk of [128, 4096]:
  - DVE  tensor_tensor_reduce: partial sums of output*masks   (inter)
  - ACT  activation(Copy, accum_out): partial sums of masks and output
  - PE   collapses partition blocks / classes at the end via tiny matmuls
"""

import os
import sys
from contextlib import ExitStack

import numpy as np

for _p in ("/opt/trn_rl_repo",):
    if _p not in sys.path and os.path.isdir(_p):
        sys.path.insert(0, _p)

import concourse.bass as bass  # noqa: E402
import concourse.tile as tile  # noqa: E402
from concourse import bacc, mybir  # noqa: E402
from concourse.bass_utils import run_bass_kernel_spmd  # noqa: E402

N_CORES = 8
B, C = 8, 4
SPATIAL = 64 * 128 * 128            # 1,048,576 per (subject, class)
P = 128                             # SBUF partitions = C * 32
COLS = (C * SPATIAL) // P           # 32768 elements per partition
# Descending chunk schedule: big DMAs (4 MiB) for bandwidth in the steady
# state, small chunks at the end so the post-last-byte compute tail is tiny.
CHUNKS = [1024, 8192, 8192, 8192, 4096, 2048, 512, 512]
BIG_FD = 4096  # chunks >= this land in the big pools, the rest in tail pools
assert sum(CHUNKS) == COLS
NCHUNK = len(CHUNKS)
FDMAX = max(CHUNKS)
EPS = 1e-7
F32 = mybir.dt.float32
BF16 = mybir.dt.bfloat16


def _dice_body(ctx: ExitStack, tc: "tile.TileContext", out_ap, x_ap, m_ap):
    nc = tc.nc
    add = mybir.AluOpType.add
    mult = mybir.AluOpType.mult
    Copy = mybir.ActivationFunctionType.Copy

    consts = ctx.enter_context(tc.tile_pool(name="consts", bufs=1))
    xpool = ctx.enter_context(tc.tile_pool(name="xin", bufs=2))
    mpool = ctx.enter_context(tc.tile_pool(name="min", bufs=2))
    xtail = ctx.enter_context(tc.tile_pool(name="xtail", bufs=3))
    mtail = ctx.enter_context(tc.tile_pool(name="mtail", bufs=3))
    small = ctx.enter_context(tc.tile_pool(name="small", bufs=1))
    psum = ctx.enter_context(tc.tile_pool(name="psum", bufs=1, space="PSUM"))

    # Block indicator: ind[q, c] = 1.0 iff q // 32 == c. lhsT for the
    # partition-block -> per-class collapse.
    ind = consts.tile([P, C], F32)
    nc.vector.memset(ind[:], 0.0)
    for c in range(C):
        nc.vector.memset(ind[c * 32 : (c + 1) * 32, c : c + 1], 1.0)
    ones4 = consts.tile([C, 1], F32)
    nc.vector.memset(ones4[:], 1.0)

    # Per-chunk partial sums (column j <- chunk j); no cross-chunk deps.
    # One fused accumulator: cols [0,N) = sum(x*m), [N,2N) = sum(m),
    # [2N,3N) = sum(x) — lets a single matmul collapse all three.
    acc = small.tile([P, 3 * NCHUNK], F32)
    # Engines must write their full elementwise result somewhere; stride-0
    # broadcast dummies avoid real [P, fd] scratch tiles (HW-verified).
    dve_dummy = small.tile([P, 1], F32)
    act_dummy = small.tile([P, 1], F32)

    off = 0
    for j, fd in enumerate(CHUNKS):
        big = fd >= BIG_FD
        xt = (xpool if big else xtail).tile([P, fd], F32, tag="xt")
        nc.sync.dma_start(out=xt[:], in_=x_ap[:, off : off + fd])
        mt = (mpool if big else mtail).tile([P, fd], F32, tag="mt")
        nc.sync.dma_start(out=mt[:], in_=m_ap[:, off : off + fd])
        off += fd

        # inter partials on DVE: out = (x*1)*m, accum = X-reduce(out).
        nc.vector.scalar_tensor_tensor(
            out=dve_dummy.broadcast_to((P, fd)),
            in0=xt[:],
            scalar=1.0,
            in1=mt[:],
            op0=mult,
            op1=mult,
            accum_out=acc[:, j : j + 1],
        )
        nc.scalar.activation(
            out=act_dummy.broadcast_to((P, fd)),
            in_=mt[:],
            func=Copy,
            accum_out=acc[:, NCHUNK + j : NCHUNK + j + 1],
        )
        # x-sum alternates between ACT and DVE so neither engine backlogs.
        if j % 2 == 0:
            nc.scalar.activation(
                out=act_dummy.broadcast_to((P, fd)),
                in_=xt[:],
                func=Copy,
                accum_out=acc[:, 2 * NCHUNK + j : 2 * NCHUNK + j + 1],
            )
        else:
            nc.vector.tensor_reduce(
                acc[:, 2 * NCHUNK + j : 2 * NCHUNK + j + 1],
                xt[:],
                axis=mybir.AxisListType.X,
                op=add,
            )

    # Partition blocks -> per-(class, quantity, chunk) sums in one matmul,
    # then one PSUM-side reduce over the chunk axis -> [4, 3].
    ps = psum.tile([C, 3 * NCHUNK], F32)
    nc.tensor.matmul(out=ps[:], lhsT=ind[:], rhs=acc[:], start=True, stop=True)

    # sums columns: 0 inter, 1 msum, 2 xsum, 3 total, 4 ms2, 5 w, 6 wi, 7 wt
    sums = small.tile([C, 8], F32)
    nc.vector.tensor_reduce(
        sums[:, 0:3],
        ps[:].rearrange("c (q n) -> c q n", q=3),
        axis=mybir.AxisListType.X,
        op=add,
    )
    nc.vector.tensor_add(out=sums[:, 3:4], in0=sums[:, 1:2], in1=sums[:, 2:3])
    # msum^2 ~ 2.7e11, so fp32 (msum^2 + 1e-7) == msum^2 exactly; skip the add.
    nc.vector.tensor_mul(out=sums[:, 4:5], in0=sums[:, 1:2], in1=sums[:, 1:2])
    nc.vector.reciprocal(out=sums[:, 5:6], in_=sums[:, 4:5])
    nc.vector.tensor_mul(out=sums[:, 6:7], in0=sums[:, 5:6], in1=sums[:, 0:1])
    nc.vector.tensor_mul(out=sums[:, 7:8], in0=sums[:, 5:6], in1=sums[:, 3:4])

    # Class sums: [1, 2] = ones4.T @ [wi, wt]  ->  (nom, sum_c w*total)
    nd_ps = psum.tile([1, 2], F32)
    nc.tensor.matmul(out=nd_ps[:], lhsT=ones4[:], rhs=sums[:, 6:8], start=True, stop=True)

    # fin columns: 0 nom, 1 den_sum, 2 denom, 3 1/denom, 4 nom/denom, 5 result
    fin = small.tile([1, 6], F32)
    nc.vector.tensor_copy(out=fin[:, 0:2], in_=nd_ps[:])
    # denom = sum_c(w*total + EPS) = den_sum + C*EPS
    nc.vector.tensor_scalar_add(out=fin[:, 2:3], in0=fin[:, 1:2], scalar1=C * EPS)
    nc.vector.reciprocal(out=fin[:, 3:4], in_=fin[:, 2:3])
    nc.vector.tensor_mul(out=fin[:, 4:5], in0=fin[:, 0:1], in1=fin[:, 3:4])
    # per-subject loss = 1 - 2 * nom/denom
    nc.vector.tensor_scalar(
        out=fin[:, 5:6], in0=fin[:, 4:5], scalar1=-2.0, scalar2=1.0, op0=mult, op1=add
    )
    nc.sync.dma_start(out=out_ap, in_=fin[:, 5:6])


_CACHE: dict[str, object] = {}


def _build():
    if "nc" in _CACHE:
        return _CACHE["nc"]
    nc = bacc.Bacc("TRN2", target_bir_lowering=False, debug=False)
    x = nc.dram_tensor("x", [P, COLS], F32, kind="ExternalInput").ap()
    m = nc.dram_tensor("m", [P, COLS], F32, kind="ExternalInput").ap()
    out = nc.dram_tensor("loss_partial", [1, 1], F32, kind="ExternalOutput").ap()
    with tile.TileContext(nc) as tc:
        with ExitStack() as ctx:
            _dice_body(ctx, tc, out, x, m)
    nc.compile()
    _CACHE["nc"] = nc
    return nc


def _in_maps(output: np.ndarray, masks: np.ndarray):
    output = np.ascontiguousarray(output, dtype=np.float32)
    masks = np.ascontiguousarray(masks, dtype=np.float32)
    return [
        {"x": output[b].reshape(P, COLS), "m": masks[b].reshape(P, COLS)}
        for b in range(N_CORES)
    ]


def run_sharded(output: np.ndarray, masks: np.ndarray, **spmd_kwargs):
    """Run the SPMD kernel; returns (loss[1], BassKernelResults)."""
    nc = _build()
    res = run_bass_kernel_spmd(
        nc, _in_maps(output, masks), list(range(N_CORES)), **spmd_kwargs
    )
    per_subj = np.array(
        [res.results[b]["loss_partial"][0, 0] for b in range(N_CORES)],
        dtype=np.float32,
    )
    loss = (per_subj.sum(dtype=np.float32) / np.float32(B)).reshape(1)
    return loss.astype(np.float32), res


def kernel(output: np.ndarray, masks: np.ndarray) -> np.ndarray:
    loss, _ = run_sharded(output, masks)
    return loss


# revision 22
# speedup vs baseline: 1.0655x; 1.0655x over previous
"""Multi-class 3D DICE loss on 8 Trainium2 NeuronCores.

Data-parallel over the subject (batch) axis: core b reduces subject b's
[C=4, 64, 128, 128] volumes to a single per-subject loss scalar; the host
averages the 8 scalars.

Per-core layout: each input tensor is viewed as [128, 32768] where
partition q = c*32 + p (class c in partition block [32c, 32c+32)).
Per chunk of [128, 4096]:
  - DVE  tensor_tensor_reduce: partial sums of output*masks   (inter)
  - ACT  activation(Copy, accum_out): partial sums of masks and output
  - PE   collapses partition blocks / classes at the end via tiny matmuls
"""

import os
import sys
from contextlib import ExitStack

import numpy as np

for _p in ("/opt/trn_rl_repo",):
    if _p not in sys.path and os.path.isdir(_p):
        sys.path.insert(0, _p)

import concourse.bass as bass  # noqa: E402
import concourse.tile as tile  # noqa: E402
from concourse import bacc, mybir  # noqa: E402
from concourse.bass_utils import run_bass_kernel_spmd  # noqa: E402

N_CORES = 8
B, C = 8, 4
SPATIAL = 64 * 128 * 128            # 1,048,576 per (subject, class)
P = 128                             # SBUF partitions = C * 32
COLS = (C * SPATIAL) // P           # 32768 elements per partition
# Descending chunk schedule: big DMAs (4 MiB) for bandwidth in the steady
# state, small chunks at the end so the post-last-byte compute tail is tiny.
CHUNKS = [8192, 8192, 8192, 4096, 2048, 1024, 512, 512]
BIG_FD = 4096  # chunks >= this land in the big pools, the rest in tail pools
assert sum(CHUNKS) == COLS
NCHUNK = len(CHUNKS)
FDMAX = max(CHUNKS)
EPS = 1e-7
F32 = mybir.dt.float32
BF16 = mybir.dt.bfloat16


def _dice_body(ctx: ExitStack, tc: "tile.TileContext", out_ap, x_ap, m_ap):
    nc = tc.nc
    add = mybir.AluOpType.add
    mult = mybir.AluOpType.mult
    Copy = mybir.ActivationFunctionType.Copy

    consts = ctx.enter_context(tc.tile_pool(name="consts", bufs=1))
    xpool = ctx.enter_context(tc.tile_pool(name="xin", bufs=2))
    mpool = ctx.enter_context(tc.tile_pool(name="min", bufs=2))
    xtail = ctx.enter_context(tc.tile_pool(name="xtail", bufs=3))
    mtail = ctx.enter_context(tc.tile_pool(name="mtail", bufs=3))
    small = ctx.enter_context(tc.tile_pool(name="small", bufs=1))
    psum = ctx.enter_context(tc.tile_pool(name="psum", bufs=1, space="PSUM"))

    # Block indicator: ind[q, c] = 1.0 iff q // 32 == c. lhsT for the
    # partition-block -> per-class collapse.
    ind = consts.tile([P, C], F32)
    nc.vector.memset(ind[:], 0.0)
    for c in range(C):
        nc.vector.memset(ind[c * 32 : (c + 1) * 32, c : c + 1], 1.0)
    ones4 = consts.tile([C, 1], F32)
    nc.vector.memset(ones4[:], 1.0)

    # Per-chunk partial sums (column j <- chunk j); no cross-chunk deps.
    # One fused accumulator: cols [0,N) = sum(x*m), [N,2N) = sum(m),
    # [2N,3N) = sum(x) — lets a single matmul collapse all three.
    acc = small.tile([P, 3 * NCHUNK], F32)
    # Engines must write their full elementwise result somewhere; stride-0
    # broadcast dummies avoid real [P, fd] scratch tiles (HW-verified).
    dve_dummy = small.tile([P, 1], F32)
    act_dummy = small.tile([P, 1], F32)

    off = 0
    for j, fd in enumerate(CHUNKS):
        big = fd >= BIG_FD
        xt = (xpool if big else xtail).tile([P, fd], F32, tag="xt")
        nc.sync.dma_start(out=xt[:], in_=x_ap[:, off : off + fd])
        mt = (mpool if big else mtail).tile([P, fd], F32, tag="mt")
        nc.sync.dma_start(out=mt[:], in_=m_ap[:, off : off + fd])
        off += fd

        # inter partials on DVE: out = (x*1)*m, accum = X-reduce(out).
        nc.vector.scalar_tensor_tensor(
            out=dve_dummy.broadcast_to((P, fd)),
            in0=xt[:],
            scalar=1.0,
            in1=mt[:],
            op0=mult,
            op1=mult,
            accum_out=acc[:, j : j + 1],
        )
        nc.scalar.activation(
            out=act_dummy.broadcast_to((P, fd)),
            in_=mt[:],
            func=Copy,
            accum_out=acc[:, NCHUNK + j : NCHUNK + j + 1],
        )
        # x-sum alternates between ACT and DVE so neither engine backlogs.
        if j % 2 == 0:
            nc.scalar.activation(
                out=act_dummy.broadcast_to((P, fd)),
                in_=xt[:],
                func=Copy,
                accum_out=acc[:, 2 * NCHUNK + j : 2 * NCHUNK + j + 1],
            )
        else:
            nc.vector.tensor_reduce(
                acc[:, 2 * NCHUNK + j : 2 * NCHUNK + j + 1],
                xt[:],
                axis=mybir.AxisListType.X,
                op=add,
            )

    # Partition blocks -> per-(class, quantity, chunk) sums in one matmul,
    # then one PSUM-side reduce over the chunk axis -> [4, 3].
    ps = psum.tile([C, 3 * NCHUNK], F32)
    nc.tensor.matmul(out=ps[:], lhsT=ind[:], rhs=acc[:], start=True, stop=True)

    # sums columns: 0 inter, 1 msum, 2 xsum, 3 total, 4 ms2, 5 w, 6 wi, 7 wt
    sums = small.tile([C, 8], F32)
    nc.vector.tensor_reduce(
        sums[:, 0:3],
        ps[:].rearrange("c (q n) -> c q n", q=3),
        axis=mybir.AxisListType.X,
        op=add,
    )
    nc.vector.tensor_add(out=sums[:, 3:4], in0=sums[:, 1:2], in1=sums[:, 2:3])
    # msum^2 ~ 2.7e11, so fp32 (msum^2 + 1e-7) == msum^2 exactly; skip the add.
    nc.vector.tensor_mul(out=sums[:, 4:5], in0=sums[:, 1:2], in1=sums[:, 1:2])
    nc.vector.reciprocal(out=sums[:, 5:6], in_=sums[:, 4:5])
    nc.vector.tensor_mul(out=sums[:, 6:7], in0=sums[:, 5:6], in1=sums[:, 0:1])
    nc.vector.tensor_mul(out=sums[:, 7:8], in0=sums[:, 5:6], in1=sums[:, 3:4])

    # Class sums: [1, 2] = ones4.T @ [wi, wt]  ->  (nom, sum_c w*total)
    nd_ps = psum.tile([1, 2], F32)
    nc.tensor.matmul(out=nd_ps[:], lhsT=ones4[:], rhs=sums[:, 6:8], start=True, stop=True)

    # fin columns: 0 nom, 1 den_sum, 2 denom, 3 1/denom, 4 nom/denom, 5 result
    fin = small.tile([1, 6], F32)
    nc.vector.tensor_copy(out=fin[:, 0:2], in_=nd_ps[:])
    # denom = sum_c(w*total + EPS) = den_sum + C*EPS
    nc.vector.tensor_scalar_add(out=fin[:, 2:3], in0=fin[:, 1:2], scalar1=C * EPS)
    nc.vector.reciprocal(out=fin[:, 3:4], in_=fin[:, 2:3])
    nc.vector.tensor_mul(out=fin[:, 4:5], in0=fin[:, 0:1], in1=fin[:, 3:4])
    # per-subject loss = 1 - 2 * nom/denom
    nc.vector.tensor_scalar(
        out=fin[:, 5:6], in0=fin[:, 4:5], scalar1=-2.0, scalar2=1.0, op0=mult, op1=add
    )
    nc.sync.dma_start(out=out_ap, in_=fin[:, 5:6])


_CACHE: dict[str, object] = {}


def _build():
    if "nc" in _CACHE:
        return _CACHE["nc"]
    nc = bacc.Bacc("TRN2", target_bir_lowering=False, debug=False)
    x = nc.dram_tensor("x", [P, COLS], F32, kind="ExternalInput").ap()
    m = nc.dram_tensor("m", [P, COLS], F32, kind="ExternalInput").ap()
    out = nc.dram_tensor("loss_partial", [1, 1], F32, kind="ExternalOutput").ap()
    with tile.TileContext(nc) as tc:
        with ExitStack() as ctx:
            _dice_body(ctx, tc, out, x, m)
    nc.compile()
    _CACHE["nc"] = nc
    return nc


def _in_maps(output: np.ndarray, masks: np.ndarray):
    output = np.ascontiguousarray(output, dtype=np.float32)
    masks = np.ascontiguousarray(masks, dtype=np.float32)
    return [
        {"x": output[b].reshape(P, COLS), "m": masks[b].reshape(P, COLS)}
        for b in range(N_CORES)
    ]


def run_sharded(output: np.ndarray, masks: np.ndarray, **spmd_kwargs):
    """Run the SPMD kernel; returns (loss[1], BassKernelResults)."""
    nc = _build()
    res = run_bass_kernel_spmd(
        nc, _in_maps(output, masks), list(range(N_CORES)), **spmd_kwargs
    )
    per_subj = np.array(
        [res.results[b]["loss_partial"][0, 0] for b in range(N_CORES)],
        dtype=np.float32,
    )
    loss = (per_subj.sum(dtype=np.float32) / np.float32(B)).reshape(1)
    return loss.astype(np.float32), res


def kernel(output: np.ndarray, masks: np.ndarray) -> np.ndarray:
    loss, _ = run_sharded(output, masks)
    return loss


# revision 23
# speedup vs baseline: 1.0740x; 1.0080x over previous
"""Multi-class 3D DICE loss on 8 Trainium2 NeuronCores.

Data-parallel over the subject (batch) axis: core b reduces subject b's
[C=4, 64, 128, 128] volumes to a single per-subject loss scalar; the host
averages the 8 scalars.

Per-core layout: each input tensor is viewed as [128, 32768] where
partition q = c*32 + p (class c in partition block [32c, 32c+32)).
Per chunk of [128, 4096]:
  - DVE  tensor_tensor_reduce: partial sums of output*masks   (inter)
  - ACT  activation(Copy, accum_out): partial sums of masks and output
  - PE   collapses partition blocks / classes at the end via tiny matmuls
"""

import os
import sys
from contextlib import ExitStack

import numpy as np

for _p in ("/opt/trn_rl_repo",):
    if _p not in sys.path and os.path.isdir(_p):
        sys.path.insert(0, _p)

import concourse.bass as bass  # noqa: E402
import concourse.tile as tile  # noqa: E402
from concourse import bacc, mybir  # noqa: E402
from concourse.bass_utils import run_bass_kernel_spmd  # noqa: E402

N_CORES = 8
B, C = 8, 4
SPATIAL = 64 * 128 * 128            # 1,048,576 per (subject, class)
P = 128                             # SBUF partitions = C * 32
COLS = (C * SPATIAL) // P           # 32768 elements per partition
# Descending chunk schedule: big DMAs (4 MiB) for bandwidth in the steady
# state, small chunks at the end so the post-last-byte compute tail is tiny.
CHUNKS = [8192, 8192, 8192, 4096, 2048, 1024, 512, 512]
BIG_FD = 4096  # chunks >= this land in the big pools, the rest in tail pools
assert sum(CHUNKS) == COLS
NCHUNK = len(CHUNKS)
FDMAX = max(CHUNKS)
EPS = 1e-7
F32 = mybir.dt.float32
BF16 = mybir.dt.bfloat16


def _dice_body(ctx: ExitStack, tc: "tile.TileContext", out_ap, x_ap, m_ap):
    nc = tc.nc
    add = mybir.AluOpType.add
    mult = mybir.AluOpType.mult
    Copy = mybir.ActivationFunctionType.Copy

    consts = ctx.enter_context(tc.tile_pool(name="consts", bufs=1))
    xpool = ctx.enter_context(tc.tile_pool(name="xin", bufs=2))
    mpool = ctx.enter_context(tc.tile_pool(name="min", bufs=2))
    xtail = ctx.enter_context(tc.tile_pool(name="xtail", bufs=3))
    mtail = ctx.enter_context(tc.tile_pool(name="mtail", bufs=3))
    small = ctx.enter_context(tc.tile_pool(name="small", bufs=1))
    psum = ctx.enter_context(tc.tile_pool(name="psum", bufs=1, space="PSUM"))

    # Block indicator: ind[q, c] = 1.0 iff q // 32 == c. lhsT for the
    # partition-block -> per-class collapse.
    ind = consts.tile([P, C], F32)
    nc.vector.memset(ind[:], 0.0)
    for c in range(C):
        nc.vector.memset(ind[c * 32 : (c + 1) * 32, c : c + 1], 1.0)
    ones4 = consts.tile([C, 1], F32)
    nc.vector.memset(ones4[:], 1.0)

    # Per-chunk partial sums (column j <- chunk j); no cross-chunk deps.
    # One fused accumulator: cols [0,N) = sum(x*m), [N,2N) = sum(m),
    # [2N,3N) = sum(x) — lets a single matmul collapse all three.
    acc = small.tile([P, 3 * NCHUNK], F32)
    # Engines must write their full elementwise result somewhere; stride-0
    # broadcast dummies avoid real [P, fd] scratch tiles (HW-verified).
    dve_dummy = small.tile([P, 1], F32)
    act_dummy = small.tile([P, 1], F32)

    off = 0
    for j, fd in enumerate(CHUNKS):
        big = fd >= BIG_FD
        xt = (xpool if big else xtail).tile([P, fd], F32, tag="xt")
        nc.sync.dma_start(out=xt[:], in_=x_ap[:, off : off + fd])
        mt = (mpool if big else mtail).tile([P, fd], F32, tag="mt")
        nc.sync.dma_start(out=mt[:], in_=m_ap[:, off : off + fd])
        off += fd

        # inter partials on DVE: out = (x*1)*m, accum = X-reduce(out).
        nc.vector.scalar_tensor_tensor(
            out=dve_dummy.broadcast_to((P, fd)),
            in0=xt[:],
            scalar=1.0,
            in1=mt[:],
            op0=mult,
            op1=mult,
            accum_out=acc[:, j : j + 1],
        )
        nc.scalar.activation(
            out=act_dummy.broadcast_to((P, fd)),
            in_=mt[:],
            func=Copy,
            accum_out=acc[:, NCHUNK + j : NCHUNK + j + 1],
        )
        # x-sum on DVE. Keep each 32 B accumulator word single-engine: cols
        # 0-7 DVE, 8-15 ACT, 16-23 DVE — mixing engines within one word
        # produced intermittent lost-update corruption on HW.
        nc.vector.tensor_reduce(
            acc[:, 2 * NCHUNK + j : 2 * NCHUNK + j + 1],
            xt[:],
            axis=mybir.AxisListType.X,
            op=add,
        )

    # Partition blocks -> per-(class, quantity, chunk) sums in one matmul,
    # then one PSUM-side reduce over the chunk axis -> [4, 3].
    ps = psum.tile([C, 3 * NCHUNK], F32)
    nc.tensor.matmul(out=ps[:], lhsT=ind[:], rhs=acc[:], start=True, stop=True)

    # sums columns: 0 inter, 1 msum, 2 xsum, 3 total, 4 ms2, 5 w, 6 wi, 7 wt
    sums = small.tile([C, 8], F32)
    nc.vector.tensor_reduce(
        sums[:, 0:3],
        ps[:].rearrange("c (q n) -> c q n", q=3),
        axis=mybir.AxisListType.X,
        op=add,
    )
    nc.vector.tensor_add(out=sums[:, 3:4], in0=sums[:, 1:2], in1=sums[:, 2:3])
    # msum^2 ~ 2.7e11, so fp32 (msum^2 + 1e-7) == msum^2 exactly; skip the add.
    nc.vector.tensor_mul(out=sums[:, 4:5], in0=sums[:, 1:2], in1=sums[:, 1:2])
    nc.vector.reciprocal(out=sums[:, 5:6], in_=sums[:, 4:5])
    nc.vector.tensor_mul(out=sums[:, 6:7], in0=sums[:, 5:6], in1=sums[:, 0:1])
    nc.vector.tensor_mul(out=sums[:, 7:8], in0=sums[:, 5:6], in1=sums[:, 3:4])

    # Class sums: [1, 2] = ones4.T @ [wi, wt]  ->  (nom, sum_c w*total)
    nd_ps = psum.tile([1, 2], F32)
    nc.tensor.matmul(out=nd_ps[:], lhsT=ones4[:], rhs=sums[:, 6:8], start=True, stop=True)

    # fin columns: 0 nom, 1 den_sum, 2 denom, 3 1/denom, 4 nom/denom, 5 result
    fin = small.tile([1, 6], F32)
    nc.vector.tensor_copy(out=fin[:, 0:2], in_=nd_ps[:])
    # denom = sum_c(w*total + EPS) = den_sum + C*EPS
    nc.vector.tensor_scalar_add(out=fin[:, 2:3], in0=fin[:, 1:2], scalar1=C * EPS)
    nc.vector.reciprocal(out=fin[:, 3:4], in_=fin[:, 2:3])
    nc.vector.tensor_mul(out=fin[:, 4:5], in0=fin[:, 0:1], in1=fin[:, 3:4])
    # per-subject loss = 1 - 2 * nom/denom
    nc.vector.tensor_scalar(
        out=fin[:, 5:6], in0=fin[:, 4:5], scalar1=-2.0, scalar2=1.0, op0=mult, op1=add
    )
    nc.sync.dma_start(out=out_ap, in_=fin[:, 5:6])


_CACHE: dict[str, object] = {}


def _build():
    if "nc" in _CACHE:
        return _CACHE["nc"]
    nc = bacc.Bacc("TRN2", target_bir_lowering=False, debug=False)
    x = nc.dram_tensor("x", [P, COLS], F32, kind="ExternalInput").ap()
    m = nc.dram_tensor("m", [P, COLS], F32, kind="ExternalInput").ap()
    out = nc.dram_tensor("loss_partial", [1, 1], F32, kind="ExternalOutput").ap()
    with tile.TileContext(nc) as tc:
        with ExitStack() as ctx:
            _dice_body(ctx, tc, out, x, m)
    nc.compile()
    _CACHE["nc"] = nc
    return nc


def _in_maps(output: np.ndarray, masks: np.ndarray):
    output = np.ascontiguousarray(output, dtype=np.float32)
    masks = np.ascontiguousarray(masks, dtype=np.float32)
    return [
        {"x": output[b].reshape(P, COLS), "m": masks[b].reshape(P, COLS)}
        for b in range(N_CORES)
    ]


def run_sharded(output: np.ndarray, masks: np.ndarray, **spmd_kwargs):
    """Run the SPMD kernel; returns (loss[1], BassKernelResults)."""
    nc = _build()
    res = run_bass_kernel_spmd(
        nc, _in_maps(output, masks), list(range(N_CORES)), **spmd_kwargs
    )
    per_subj = np.array(
        [res.results[b]["loss_partial"][0, 0] for b in range(N_CORES)],
        dtype=np.float32,
    )
    loss = (per_subj.sum(dtype=np.float32) / np.float32(B)).reshape(1)
    return loss.astype(np.float32), res


def kernel(output: np.ndarray, masks: np.ndarray) -> np.ndarray:
    loss, _ = run_sharded(output, masks)
    return loss
